# revision 36
# baseline (speedup 1.0000x reference)
"""AttentionAggregate kernel for 8 TRN2 NeuronCores (Bass/Tile).

Problem: out, token_sizes = AttentionAggregate(x, clusters, Wq, Wkv, Wout)
  B=2, N=8192, NC=512, D=512, H=16, HD=32; softmax over the cluster axis.

Sharding: 8 cores = 2 batches x 4 token-shards (2048 tokens each). Each core
projects q/k/v for its tokens (all 16 heads), computes exp(scores) with the
per-token softmax denominator folded into a scaled v (plus a ones-column that
yields token_sizes), and accumulates partial O_aug = v_scaled^T @ E in PSUM.
A ReduceScatter over each batch's 4 cores sums the partials and hands every
core one 128-cluster block, which it normalizes by token_sizes and projects
through Wout. The host only concatenates the 8 disjoint output slices.

Matmuls run in float32r (TF32-like, ~1.6e-4 rel err, full PE rate at N>=256);
softmax exp runs on the scalar engine in fp32 over head-pairs, and the
per-token denominators come from a DVE pass with a fused row-sum accumulator
(keeping the busiest engine, ScalarE, near its 128us exp floor).
"""
import sys

sys.path.insert(0, "/opt/trn_rl_repo")

import numpy as np
from contextlib import ExitStack

import concourse.bass as bass
import concourse.tile as tile
from concourse import bacc, mybir
from concourse.bass_utils import run_bass_kernel_spmd
from concourse.masks import make_identity

F32 = mybir.dt.float32
F32R = mybir.dt.float32r
AF = mybir.ActivationFunctionType

B, N, NC, D, H = 2, 8192, 512, 512, 16
HD = D // H                # 32
SCALE = HD ** -0.5
EPS = 1e-6
TOK = N // 4               # 2048 tokens per core
NT = TOK // 128            # 16 token tiles
NCH = TOK // 512           # 4 chunks of 512 tokens
VW = 34                    # per-head stride in v layout (32 v + 1 one + 1 pad)


def _build_nc(loop=1, fake_collective=False, phases="abcd"):
    nc = bacc.Bacc("TRN2", target_bir_lowering=False, debug=False, num_devices=8)

    xs = nc.declare_dram_parameter("xs", [TOK, D], F32, isOutput=False)
    cl = nc.declare_dram_parameter("cl", [NC, D], F32, isOutput=False)
    wq = nc.declare_dram_parameter("wq", [D, D], F32, isOutput=False)
    wkv = nc.declare_dram_parameter("wkv", [D, 2 * D], F32, isOutput=False)
    wout = nc.declare_dram_parameter("wout", [D, D], F32, isOutput=False)
    out_slice = nc.declare_dram_parameter("out_slice", [128, D], F32, isOutput=True)
    ts_slice = nc.declare_dram_parameter("ts_slice", [128, 1], F32, isOutput=True)


    with ExitStack() as ctx:
        tc = ctx.enter_context(tile.TileContext(nc))
        const = ctx.enter_context(tc.tile_pool(name="const", bufs=1))
        wts = ctx.enter_context(tc.tile_pool(name="wts", bufs=1))
        stage = ctx.enter_context(tc.tile_pool(name="stage", bufs=1))
        work = ctx.enter_context(tc.tile_pool(name="work", bufs=2))
        xTp = ctx.enter_context(tc.tile_pool(name="xTp", bufs=2))
        dpool = ctx.enter_context(tc.tile_pool(name="dpool", bufs=2))
        drams = ctx.enter_context(tc.tile_pool(name="drams", bufs=2, space="DRAM"))

        ident = const.tile([128, 128], F32)
        make_identity(nc, ident)
        ones_sb = const.tile([128, H], F32)
        nc.vector.memset(ones_sb, 1.0)

        # ---- persistent fp32r operands (rounded on the DVE copy) ----
        wkv_r = wts.tile([128, 4, 2 * D], F32R)   # [dpart, dslice, col]
        wq_r = wts.tile([128, 4, D], F32R)
        wout_r = wts.tile([128, 4, D], F32R)
        qT_r = wts.tile([128, 4, NC], F32R)       # rows h*HD+hd (slice g = heads 4g..4g+3)
        kT_r = wts.tile([128, 4, TOK], F32R)      # rows within group, [grp, token]
        v_r = wts.tile([128, NT, H, VW], F32R)    # per tile/head: 32 v | 1 one | 1 pad
        clT_r = wts.tile([128, 4, NC], F32R)

        for it in range(loop):
            # =============== phase A: weights, qT, kT, v ===============
            for k in range(4):
                st = stage.tile([128, 2 * D], F32, tag="wst")
                nc.sync.dma_start(out=st, in_=wkv[128 * k:128 * k + 128, :])
                nc.vector.tensor_copy(wkv_r[:, k, :], st)
                st2 = stage.tile([128, D], F32, tag="wst2")
                nc.sync.dma_start(out=st2, in_=wq[128 * k:128 * k + 128, :])
                nc.vector.tensor_copy(wq_r[:, k, :], st2)
                st3 = stage.tile([128, D], F32, tag="wst3")
                nc.sync.dma_start(out=st3, in_=wout[128 * k:128 * k + 128, :])
                nc.vector.tensor_copy(wout_r[:, k, :], st3)

            with tc.tile_pool(name="psA", bufs=4, space="PSUM") as psA:
                # clusters^T then qT = Wq^T @ clusters^T
                for t in range(4):
                    cl_sb = work.tile([128, D], F32, tag="cl")
                    nc.sync.dma_start(out=cl_sb, in_=cl[128 * t:128 * t + 128, :])
                    trp = psA.tile([128, 512], F32, tag="txp")
                    for j in range(4):
                        nc.tensor.transpose(
                            trp[:, 128 * j:128 * j + 128],
                            cl_sb[:, 128 * j:128 * j + 128], ident)
                    nc.scalar.copy(clT_r[:, :, 128 * t:128 * t + 128], trp)
                for g in range(4):
                    qp = psA.tile([128, NC], F32, tag="proj")
                    for k in range(4):
                        nc.tensor.matmul(
                            qp, wq_r[:, k, 128 * g:128 * g + 128], clT_r[:, k, :],
                            start=(k == 0), stop=(k == 3))
                    nc.scalar.copy(qT_r[:, g, :], qp)

                # x^T per 512-token chunk, then kT and v projections
                for ch in range(NCH):
                    xT_sb = xTp.tile([128, 4, 512], F32R, tag="xT")
                    x_sb = work.tile([128, 4, D], F32, tag="x")
                    nc.sync.dma_start(
                        out=x_sb,
                        in_=xs[512 * ch:512 * ch + 512, :].rearrange(
                            "(t p) d -> p t d", p=128))
                    for t in range(4):
                        trp = psA.tile([128, 512], F32, tag="txp")
                        for j in range(4):
                            nc.tensor.transpose(
                                trp[:, 128 * j:128 * j + 128],
                                x_sb[:, t, 128 * j:128 * j + 128], ident)
                        nc.scalar.copy(xT_sb[:, :, 128 * t:128 * t + 128], trp)
                    for g in range(4):
                        kp = psA.tile([128, 512], F32, tag="proj")
                        for k in range(4):
                            nc.tensor.matmul(
                                kp, wkv_r[:, k, 128 * g:128 * g + 128], xT_sb[:, k, :],
                                start=(k == 0), stop=(k == 3))
                        nc.scalar.copy(
                            kT_r[:, g, 512 * ch:512 * ch + 512], kp)
                    for t in range(4):
                        vp = psA.tile([128, 512], F32, tag="proj")
                        for k in range(4):
                            nc.tensor.matmul(
                                vp, xT_sb[:, k, 128 * t:128 * t + 128],
                                wkv_r[:, k, D:2 * D],
                                start=(k == 0), stop=(k == 3))
                        tt = 4 * ch + t
                        nc.vector.tensor_copy(v_r[:, tt, :, 0:HD], vp)
                        nc.vector.tensor_copy(v_r[:, tt, :, HD:HD + 1], ones_sb)

            # =============== phase B: attention ===============
            if "b" not in phases:
                continue
            with (
                tc.tile_pool(name="psSC", bufs=2, space="PSUM") as psSC,
                tc.tile_pool(name="psO", bufs=1, space="PSUM") as psO,
                tc.tile_pool(name="expp", bufs=5) as expp,
                tc.tile_pool(name="smallp", bufs=16) as smallp,
                tc.tile_pool(name="osb", bufs=2) as osb,
            ):
                oaug_reds = []
                for g in range(4):
                    o_ps0 = psO.tile([128, 512], F32, tag="o0")
                    o_ps1 = psO.tile([128, 512], F32, tag="o1")
                    o_ps2 = psO.tile([128, 512], F32, tag="o2")
                    o_ps3 = psO.tile([128, 512], F32, tag="o3")
                    o_ps = [o_ps0, o_ps1, o_ps2, o_ps3]
                    for t in range(NT):
                        halves = []
                        for half in range(2):
                            sc = psSC.tile([128, 1024], F32, tag="sc")
                            for i2 in range(2):
                                i = 2 * half + i2
                                nc.tensor.matmul(
                                    sc[:, 512 * i2:512 * i2 + 512],
                                    kT_r[32 * i:32 * i + 32, g,
                                         128 * t:128 * t + 128],
                                    qT_r[32 * i:32 * i + 32, g, :],
                                    start=True, stop=True,
                                    tile_position=(32 * i, 0))
                            halves.append(sc)
                        es = []
                        for half in range(2):
                            e2 = expp.tile([128, 1024], F32R, tag="exp")
                            nc.scalar.activation(
                                out=e2, in_=halves[half], func=AF.Exp, scale=SCALE)
                            es.append(e2)
                        for i in range(4):
                            half, i2 = divmod(i, 2)
                            h = 4 * g + i
                            e = es[half][:, 512 * i2:512 * i2 + 512]
                            den = smallp.tile([128, 1], F32, tag="den")
                            nc.vector.tensor_scalar(
                                out=e, in0=e, scalar1=1.0, scalar2=0.0,
                                op0=mybir.AluOpType.mult,
                                op1=mybir.AluOpType.add, accum_out=den)
                            inv = smallp.tile([128, 1], F32, tag="inv")
                            nc.vector.reciprocal(inv, den)
                            vsc = smallp.tile([128, HD + 1], F32R, tag="vs")
                            nc.vector.tensor_scalar_mul(
                                vsc, v_r[:, t, h, 0:HD + 1], inv)
                            nc.tensor.matmul(
                                o_ps[i][0:HD + 1, :], vsc, e,
                                start=(t == 0), stop=(t == NT - 1))
                    # phase C: dump this group's O_aug partials + reduce-scatter
                    oaug_part = drams.tile([4, 4, HD + 1, 128], F32, tag="opart")
                    oaug_red = drams.tile([4, HD + 1, 128], F32, tag="ored")
                    for i in range(4):
                        o_sb = osb.tile([HD + 1, 512], F32, tag="osb")
                        nc.vector.tensor_copy(o_sb, o_ps[i][0:HD + 1, :])
                        nc.sync.dma_start(
                            out=oaug_part[:, i].rearrange("b a j -> a b j"),
                            in_=o_sb)
                    if fake_collective:
                        nc.sync.dma_start(out=oaug_red[:], in_=oaug_part[0])
                    else:
                        nc.gpsimd.collective_compute(
                            "ReduceScatter", mybir.AluOpType.add,
                            replica_groups=[[0, 1, 2, 3], [4, 5, 6, 7]],
                            ins=[oaug_part.opt()],
                            outs=[oaug_red.opt()],
                        )
                    oaug_reds.append(oaug_red)

            if "d" not in phases:
                continue

            # =============== phase D: normalize + output projection ===============
            with (
                tc.tile_pool(name="psD", bufs=2, space="PSUM") as psD,
                tc.tile_pool(name="psDo", bufs=1, space="PSUM") as psDo,
                tc.tile_pool(name="dwork", bufs=4) as dwork,
            ):
                ostack = dpool.tile([128, 4, 128], F32R, tag="ostack")
                ts_acc = dwork.tile([128, 1], F32, tag="tsacc")
                nc.vector.memset(ts_acc, 0.0)
                for g in range(4):
                    o_red_sb = dpool.tile([HD + 1, 4 * 128], F32, tag="ored")
                    nc.sync.dma_start(
                        out=o_red_sb,
                        in_=oaug_reds[g][:].rearrange("h a j -> a h j"))
                    for i in range(4):
                        h = 4 * g + i
                        trp = psD.tile([128, HD + 1], F32, tag="tr1")
                        nc.tensor.transpose(
                            trp, o_red_sb[:, 128 * i:128 * i + 128],
                            ident[:HD + 1, :HD + 1])
                        onat = dwork.tile([128, HD + 1], F32, tag="onat")
                        nc.scalar.copy(onat, trp)
                        nc.vector.tensor_add(ts_acc, ts_acc, onat[:, HD:HD + 1])
                        tseps = dwork.tile([128, 1], F32, tag="tseps")
                        nc.vector.tensor_scalar_add(tseps, onat[:, HD:HD + 1], EPS)
                        inv2 = dwork.tile([128, 1], F32, tag="inv2")
                        nc.vector.reciprocal(inv2, tseps)
                        osc = dwork.tile([128, HD], F32, tag="osc")
                        nc.vector.tensor_scalar_mul(osc, onat[:, :HD], inv2)
                        trp2 = psD.tile([HD, 128], F32, tag="tr2")
                        nc.tensor.transpose(trp2, osc, ident)
                        k, i2 = divmod(h, 4)
                        nc.scalar.copy(ostack[32 * i2:32 * i2 + 32, k, :], trp2)
                tsm = dwork.tile([128, 1], F32, tag="tsm")
                nc.vector.tensor_scalar_mul(tsm, ts_acc, 1.0 / H)
                nc.sync.dma_start(out=ts_slice[:], in_=tsm)

                op = psDo.tile([128, D], F32, tag="outp")
                for k in range(4):
                    nc.tensor.matmul(
                        op, ostack[:, k, :], wout_r[:, k, :],
                        start=(k == 0), stop=(k == 3))
                out_sb = dpool.tile([128, D], F32, tag="osl")
                nc.vector.tensor_copy(out_sb, op)
                nc.sync.dma_start(out=out_slice[:], in_=out_sb)

    nc.compile()
    return nc


_NC_CACHE = {}


def _get_nc(loop=1):
    if loop not in _NC_CACHE:
        _NC_CACHE[loop] = _build_nc(loop)
    return _NC_CACHE[loop]


def _shard_inputs(x, clusters, Wq, Wkv, Wout):
    in_maps = []
    for c in range(8):
        b, s = c // 4, c % 4
        in_maps.append({
            "xs": np.ascontiguousarray(x[b, s * TOK:(s + 1) * TOK]),
            "cl": np.ascontiguousarray(clusters[b]),
            "wq": np.asarray(Wq),
            "wkv": np.asarray(Wkv),
            "wout": np.asarray(Wout),
        })
    return in_maps


def kernel(x, clusters, Wq, Wkv, Wout):
    x = np.asarray(x, dtype=np.float32)
    clusters = np.asarray(clusters, dtype=np.float32)
    Wq = np.asarray(Wq, dtype=np.float32)
    Wkv = np.asarray(Wkv, dtype=np.float32)
    Wout = np.asarray(Wout, dtype=np.float32)

    nc = _get_nc()
    in_maps = _shard_inputs(x, clusters, Wq, Wkv, Wout)
    res = run_bass_kernel_spmd(nc, in_maps, list(range(8)))

    out = np.zeros((B, NC, D), np.float32)
    ts = np.zeros((B, NC), np.float32)
    for c in range(8):
        b, s = c // 4, c % 4
        out[b, 128 * s:128 * s + 128] = res.results[c]["out_slice"]
        ts[b, 128 * s:128 * s + 128] = res.results[c]["ts_slice"][:, 0]
    return out, ts


if __name__ == "__main__":
    _get_nc()
    print("kernel build+compile OK")
